# revision 26
# baseline (speedup 1.0000x reference)
"""Trainium2 Bass kernel for a pre-LN transformer block (B=2,T=2048,C=768,H=12,F=3072).

Sharding: pure data-parallel over 8 cores = 2 batches x 4 query-groups. Every
core runs an identical SPMD program; per-core differences are carried by data:
the host sends each core a row-PERMUTED copy of its batch's x so that the
core's own query tiles sit at fixed positions (first 128 rows of each 512-row
chunk), and causality is applied via a per-core mask tensor (attention is
permutation-invariant over keys).

fp8(e4m3) DoubleRow matmuls (0.5 cy/col, 256-deep contraction) for QKV
projections, AV, out-proj and both MLP linears; S stays bf16 (contraction is
only D=64, DoubleRow inapplicable). Activations are scaled by powers of two
into e4m3's sweet spot and descaled exactly in the PSUM->SBUF copies. Softmax:
no max-subtraction (|S|<2), e' = exp(S + ln 64) written fp8 by the Act engine,
denominator via an s_v-valued extra V column, normalization deferred to the
per-head [64,512] output. LN is split: stats on DVE (bn_stats), normalize on
Act with per-partition scale/bias. Weights are host-pre-tiled fp8 so each load
is one contiguous-per-partition DMA.
"""
import sys

sys.path.insert(0, "/opt/trn_rl_repo")
sys.path.insert(0, "/opt/trn_rl_repo/concourse")

from contextlib import ExitStack

import numpy as np

import concourse.bass as bass
import concourse.tile as tile
from concourse import bacc, mybir
from concourse.bass_utils import run_bass_kernel_spmd
from concourse.masks import make_identity

B, T, C, H, D, F = 2, 2048, 768, 12, 64, 3072
EPS = 1e-5
NCORES = 8
QUAD = 4          # cores per batch
NJ = 4            # q-tiles of 128 per core
R = 512           # rows per core
NRT = T // 128    # 16 row tiles of x_full
NCB = C // 128    # 6 feature chunks
NP = NCB // 2     # 3 contraction pairs (DoubleRow)
NFT = F // 128    # 24 mlp feature chunks
NFP = NFT // 2    # 12 mlp feature pairs

F32 = mybir.dt.float32
F32R = mybir.dt.float32r
BF16 = mybir.dt.bfloat16
FP8 = mybir.dt.float8e4

# power-of-two activation scales (see docstring)
S_Z = 16.0        # z = LN1 out (pre-gain)
S_Z2 = 16.0       # z2 = LN2 out
S_E = 64.0        # e = exp(S) * S_E  (folded into exp bias)
S_V = 32.0        # v
S_A = 32.0        # attention out
S_A1 = 16.0       # mlp hidden relu out
LN64 = float(np.log(S_E))


def build_program(with_cv=True):
    nc = bacc.Bacc("TRN2", target_bir_lowering=False, debug=False,
                   num_devices=NCORES)
    # ---- DRAM I/O ----
    x_full = nc.dram_tensor("x_full", (T, C), F32, kind="ExternalInput").ap()
    msk_d = nc.dram_tensor("msk", (128, 512), FP8, kind="ExternalInput").ap()
    wq_d = nc.dram_tensor("wq", (128, NCB * 768), FP8, kind="ExternalInput").ap()
    wk_d = nc.dram_tensor("wk", (128, NCB * 768), FP8, kind="ExternalInput").ap()
    wv_d = nc.dram_tensor("wv", (128, NP * 1536), FP8, kind="ExternalInput").ap()
    wp_d = nc.dram_tensor("wp", (128, NP * 1536), FP8, kind="ExternalInput").ap()
    cqk_d = nc.dram_tensor("cqk", (128, 12), F32, kind="ExternalInput").ap()
    # per hf half [1, 480]: cols 0:384 = v-bias * (S_Z*2^12); cols 384:480 =
    # per-head [65536, 0*15] pattern -> vrm8 denominator col (S_V) + zero pad
    cv_d = nc.dram_tensor("cv", (1, 960), F32R, kind="ExternalInput").ap()
    bp_d = nc.dram_tensor("bp", (1, C), F32R, kind="ExternalInput").ap()
    w1_d = nc.dram_tensor("w1", (128, NFT * 768), FP8, kind="ExternalInput").ap()
    c1_d = nc.dram_tensor("c1", (128, NFT), F32, kind="ExternalInput").ap()
    w2_d = nc.dram_tensor("w2", (128, NCB * 3072), FP8, kind="ExternalInput").ap()
    b2c_d = nc.dram_tensor("b2c", (128, NCB), F32, kind="ExternalInput").ap()
    ones_d = nc.dram_tensor("ones1", (1, 512), F32R, kind="ExternalInput").ap()
    c32_d = nc.dram_tensor("c32", (1, 64), F32R, kind="ExternalInput").ap()
    out_d = nc.dram_tensor("out", (R, C), F32, kind="ExternalOutput").ap()

    Exp = mybir.ActivationFunctionType.Exp
    Relu = mybir.ActivationFunctionType.Relu
    Ident = mybir.ActivationFunctionType.Identity
    Sqrt = mybir.ActivationFunctionType.Sqrt
    DR = mybir.MatmulPerfMode.DoubleRow
    MUL = mybir.AluOpType.mult
    ADD = mybir.AluOpType.add

    with tile.TileContext(nc) as tc, ExitStack() as top:
        const = top.enter_context(tc.tile_pool(name="const", bufs=1))
        ident_bf = const.tile([128, 128], BF16)
        make_identity(nc, ident_bf[:])
        ident_f = const.tile([128, 128], F32)
        make_identity(nc, ident_f[:])
        c32 = const.tile([1, 64], F32R)
        epsc = const.tile([128, 1], F32)
        nc.vector.memset(epsc[:], EPS / (S_Z * S_Z))
        ln64c = const.tile([128, 1], F32)
        nc.vector.memset(ln64c[:], LN64)
        ones = const.tile([1, 512], F32R)
        msk = const.tile([128, 512], FP8)
        cqk = const.tile([128, 12], F32)
        cv = const.tile([1, 960], F32R)
        bp = const.tile([1, C], F32R)
        c1 = const.tile([128, NFT], F32)
        b2c = const.tile([128, NCB], F32)

        def load_consts():
            nc.sync.dma_start(ones[:], ones_d)
            nc.sync.dma_start(c32[:], c32_d)
            nc.sync.dma_start(msk[:], msk_d)
            nc.sync.dma_start(cqk[:], cqk_d)
            nc.sync.dma_start(cv[:], cv_d)
            nc.sync.dma_start(bp[:], bp_d)
            nc.sync.dma_start(c1[:], c1_d)
            nc.sync.dma_start(b2c[:], b2c_d)

        # persistent tiles
        act = top.enter_context(tc.tile_pool(name="act", bufs=1))
        xo_sb = [act.tile([128, C], F32, tag=f"xo{j}", name=f"xo{j}") for j in range(NJ)]
        x2 = [act.tile([128, C], F32, tag=f"x2{j}", name=f"x2{j}") for j in range(NJ)]

        z2pool = top.enter_context(tc.tile_pool(name="z2p", bufs=1))
        # z2 fp8 pairs: [128, 2*512] with halves = cb even/odd of pair pp
        z2f8 = [z2pool.tile([128, 1024], FP8, tag=f"z2{pp}", name=f"z2{pp}")
                for pp in range(NP)]

        stats = top.enter_context(tc.tile_pool(name="stats", bufs=3))

        # attention-lifetime tensors (released after proj)
        kvat = tc.alloc_tile_pool(name="kvat", bufs=1)
        # z fp8 pairs: [128, 2*2048]
        zfm8 = [kvat.tile([128, 4096], FP8, tag=f"zf{pp}", name=f"zf{pp}")
                for pp in range(NP)]
        qfm = [kvat.tile([128, R], BF16, tag=f"qf{ct}", name=f"qf{ct}")
               for ct in range(NCB)]
        kfm = [[kvat.tile([128, 512], BF16, tag=f"kf{ct}_{rc}", name=f"kf{ct}_{rc}")
                for rc in range(4)] for ct in range(NCB)]
        # v fp8: per rt-pair [128, 2*(H*80)]; head blocks padded 65->80 so the
        # DoubleRow pair stride (H*80) and head offsets (80h) are 16-aligned.
        # col 64 of each head block = S_V (softmax denominator); 65:80 pad.
        VB = 80
        vrm8 = [kvat.tile([128, 2 * H * VB], FP8, tag=f"vr{rp}", name=f"vr{rp}")
                for rp in range(NRT // 2)]
        # attention out fp8 pairs: [128, 2*512]
        afm8 = [kvat.tile([128, 1024], FP8, tag=f"af{pp}", name=f"af{pp}")
                for pp in range(NP)]

        def ln_tile(x_ap, sz):
            """LN stats on DVE, normalize on Act; returns bf16 z*sz tile."""
            st = stats.tile([128, 12], F32, tag="lnst")
            nc.vector.bn_stats(st[:, 0:6], x_ap[:, 0:384])
            nc.vector.bn_stats(st[:, 6:12], x_ap[:, 384:768])
            mv = stats.tile([128, 2], F32, tag="lnmv")
            nc.vector.bn_aggr(mv[:], st[:].rearrange("p (g k) -> p g k", g=2))
            sd = stats.tile([128, 1], F32, tag="lnsd")
            # sd = sqrt(var + eps) / sz;  rr = sz / sqrt(var + eps)
            nc.scalar.activation(sd[:], mv[:, 1:2], Sqrt,
                                 bias=epsc[:], scale=1.0 / (sz * sz))
            rr = stats.tile([128, 1], F32, tag="lnrr")
            nc.vector.reciprocal(rr[:], sd[:])
            nmr = stats.tile([128, 1], F32, tag="lnnm")
            nc.vector.tensor_scalar(nmr[:], mv[:, 0:1], -1.0, rr[:],
                                    op0=MUL, op1=MUL)
            zt = stats.tile([128, C], BF16, tag="lnz", bufs=2)
            nc.scalar.activation(zt[:], x_ap, Ident, bias=nmr[:], scale=rr[:])
            return zt

        with ExitStack() as phase1:
            ld = phase1.enter_context(tc.tile_pool(name="ld", bufs=2))
            wst = phase1.enter_context(tc.tile_pool(name="wst", bufs=2))
            wvp_ = phase1.enter_context(tc.tile_pool(name="wvh", bufs=1))
            tp = phase1.enter_context(tc.tile_pool(name="tp", bufs=2, space="PSUM"))
            kqp = phase1.enter_context(tc.tile_pool(name="kqp", bufs=2, space="PSUM"))
            vp_ = phase1.enter_context(tc.tile_pool(name="vp", bufs=2, space="PSUM"))

            # wv fp8 (moving operand layout), one DMA
            wv8 = wvp_.tile([128, NP * 1536], FP8, name="wv8")
            nc.sync.dma_start(wv8[:], wv_d)

            def emit_q():
                for ct in range(NCB):
                    wqg = wst.tile([128, NP * 256], FP8, tag="wq",
                                   name=f"wqg{ct}", bufs=2)
                    nc.sync.dma_start(wqg[:], wq_d[:, 768 * ct:768 * ct + 768])
                    qp = kqp.tile([128, 512], F32, tag="qp")
                    for cb in range(NCB):
                        # non-DR fp8 (FD=128 per j would kill DoubleRow; FWL
                        # runs fp8 at bf16 speed with FD=512 here)
                        rhs = zfm8[cb // 2][:].rearrange(
                            "p (i j u) -> p i j u", i=2, u=512)[
                            :, cb % 2, :, 0:128]
                        nc.tensor.matmul(
                            qp[:].rearrange("p (j u) -> p j u", j=NJ),
                            wqg[:, 128 * cb:128 * cb + 128], rhs,
                            start=(cb == 0), stop=(cb == NCB - 1),
                            skip_group_check=True)
                    nc.vector.tensor_scalar(
                        qfm[ct][:], qp[:], 1.0 / (S_Z * 2 ** 17),
                        cqk[:, ct:ct + 1], op0=MUL, op1=ADD)

            def emit_k(ct, rc):
                wkg = wst.tile([128, NP * 256], FP8, tag="wk",
                               name=f"wkg{ct}_{rc}", bufs=2)
                nc.sync.dma_start(wkg[:], wk_d[:, 768 * ct:768 * ct + 768])
                kp = kqp.tile([128, 512], F32, tag="kp")
                for pp in range(NP):
                    lhsT = wkg[:, 256 * pp:256 * pp + 256].rearrange(
                        "p (i m) -> p i m", i=2)
                    rhs = zfm8[pp][:].rearrange(
                        "p (i t) -> p i t", i=2)[:, :, 512 * rc:512 * rc + 512]
                    nc.tensor.matmul(kp[:], lhsT, rhs,
                                     start=(pp == 0), stop=(pp == NP - 1),
                                     perf_mode=DR)
                if (ct + rc) % 2 == 0:
                    nc.vector.tensor_scalar(
                        kfm[ct][rc][:], kp[:], 1.0 / (S_Z * 2 ** 12),
                        cqk[:, 6 + ct:7 + ct], op0=MUL, op1=ADD)
                else:
                    nc.scalar.activation(
                        kfm[ct][rc][:], kp[:], Ident,
                        bias=cqk[:, 6 + ct:7 + ct], scale=1.0 / (S_Z * 2 ** 12))

            def emit_v(rt):
                for hf in range(2):
                    vp = vp_.tile([128, 480], F32, tag="vp")
                    for pp in range(NP):
                        lhsT = zfm8[pp][:].rearrange(
                            "p (i t) -> p i t", i=2)[:, :, 128 * rt:128 * rt + 128]
                        rhs = wv8[:, 1536 * pp:1536 * pp + 1536].rearrange(
                            "p (i f) -> p i f", i=2)[:, :, 384 * hf:384 * hf + 384]
                        nc.tensor.matmul(vp[:, 0:384], lhsT, rhs,
                                         start=(pp == 0), stop=False,
                                         perf_mode=DR, skip_group_check=True)
                    # bias + denominator-column/pad pattern
                    nc.tensor.matmul(vp[:], ones[0:1, 0:128],
                                     cv[0:1, 480 * hf:480 * hf + 480],
                                     start=False, stop=True,
                                     skip_group_check=True)
                    vr = vrm8[rt // 2][:].rearrange(
                        "p (i h k) -> p i h k", i=2, k=VB)
                    nc.vector.tensor_scalar(
                        vr[:, rt % 2, 6 * hf:6 * hf + 6, 0:64],
                        vp[:, 0:384].rearrange("p (h k) -> p h k", k=64),
                        S_V / (S_Z * 2 ** 12), None, op0=MUL)
                    nc.vector.tensor_scalar(
                        vr[:, rt % 2, 6 * hf:6 * hf + 6, 64:VB],
                        vp[:, 384:480].rearrange("p (h k) -> p h k", k=16),
                        S_V / (S_Z * 2 ** 12), None, op0=MUL)

            # ---- Stage A: x load + LN1 + transpose -> zfm8; interleave Q/K/V ----
            load_consts()
            rt_order = [0, 4, 8, 12] + [rt for rt in range(NRT) if rt % 4 != 0]
            done = set()
            for idx, rt in enumerate(rt_order):
                xh = ld.tile([128, C], F32, tag="xf", name=f"xh{rt}", bufs=2)
                nc.sync.dma_start(xh[:], x_full[128 * rt:128 * rt + 128, :])
                if idx == 4:
                    emit_q()
                zt = ln_tile(xh[:], S_Z)
                for cp in range(NP):
                    pt = tp.tile([128, 256], BF16, tag="zt")
                    for u in range(2):
                        cb = 2 * cp + u
                        nc.tensor.transpose(pt[:, 128 * u:128 * u + 128],
                                            zt[:, 128 * cb:128 * cb + 128],
                                            ident_bf[:])
                    for u in range(2):
                        eng = nc.vector if (cp + u) % 2 == 0 else nc.scalar
                        if eng is nc.scalar:
                            nc.scalar.activation(
                                zfm8[cp][:, 2048 * u + 128 * rt:
                                          2048 * u + 128 * rt + 128],
                                pt[:, 128 * u:128 * u + 128], Ident)
                        else:
                            nc.vector.tensor_copy(
                                zfm8[cp][:, 2048 * u + 128 * rt:
                                          2048 * u + 128 * rt + 128],
                                pt[:, 128 * u:128 * u + 128])
                done.add(rt)
                emit_v(rt)
                for rc in range(4):
                    if all(4 * rc + k in done for k in range(4)) and \
                            (rc, "k") not in done:
                        done.add((rc, "k"))
                        for ct in range(NCB):
                            emit_k(ct, rc)

            # own x rows for the residual
            for j in range(NJ):
                nc.sync.dma_start(xo_sb[j][:], x_full[512 * j:512 * j + 128, :])

        # ---- prefetch proj weights during attention (DMA idle there) ----
        wpp = tc.alloc_tile_pool(name="wpp", bufs=1)
        wp8 = wpp.tile([128, NP * 1536], FP8, name="wp8")
        nc.sync.dma_start(wp8[:], wp_d)

        # ---- Stage C: attention (S^T sweep, bf16 S + fp8 DoubleRow AV) ----
        with ExitStack() as phase2:
            ep = phase2.enter_context(tc.tile_pool(name="ep", bufs=2))
            sp_ = phase2.enter_context(tc.tile_pool(name="sp", bufs=3, space="PSUM"))
            app = phase2.enter_context(tc.tile_pool(name="app", bufs=1, space="PSUM"))
            bcp = phase2.enter_context(tc.tile_pool(name="bcp", bufs=1, space="PSUM"))
            for h in range(H):
                hb, ho = h // 2, 64 * (h % 2)
                ap = app.tile([128, 512], F32, tag="ap")
                es = []
                for c in range(4):
                    n = 512 - 128 * c
                    for pr in range(2):
                        sp = sp_.tile([128, 1024], F32, tag="sp")
                        for hf in range(2):
                            kb = 2 * pr + hf
                            nc.tensor.matmul(
                                sp[:, 512 * hf:512 * hf + n],
                                kfm[hb][c][ho:ho + 64, 128 * kb:128 * kb + 128],
                                qfm[hb][ho:ho + 64, 128 * c: 512],
                                start=True, stop=True)
                        e = ep.tile([128, 1024], FP8, tag="e", bufs=16)
                        nc.scalar.activation(
                            e[:].rearrange("p (b n) -> p b n", b=2)[:, :, 0:n],
                            sp[:].rearrange("p (b n) -> p b n", b=2)[:, :, 0:n],
                            Exp, bias=ln64c[:])
                        nc.vector.tensor_tensor(
                            e[:].rearrange("p (b n) -> p b n", b=2)[:, :, 0:128],
                            e[:].rearrange("p (b n) -> p b n", b=2)[:, :, 0:128],
                            msk[:, 256 * pr:256 * pr + 256]
                                .rearrange("p (b n) -> p b n", b=2), op=MUL)
                        es.append((c, n, pr, e))
                for c, n, pr, e in es:
                    nc.tensor.matmul(
                        ap[0:VB, 128 * c:512],
                        vrm8[2 * c + pr][:].rearrange(
                            "p (i f) -> p i f", i=2)[:, :, VB * h:VB * h + VB],
                        e[:].rearrange("p (b n) -> p b n", b=2)[:, :, 0:n],
                        start=(c == 0 and pr == 0), stop=(c == 3 and pr == 1),
                        perf_mode=DR, skip_group_check=True)
                invd = ep.tile([1, 512], F32R, tag="invd")
                with nc.allow_low_precision(reason="fp32r invd for broadcast mm"):
                    nc.vector.reciprocal(invd[:], ap[64:65, :])
                bc = bcp.tile([128, 512], F32, tag="bc")
                nc.tensor.matmul(bc[0:64, :], c32[0:1, 0:64], invd[:],
                                 start=True, stop=True)
                raw = ep.tile([64, 512], F32, tag="raw")
                nc.vector.tensor_copy(raw[:], ap[0:64, :])
                pp_, i_ = h // 4, (h // 2) % 2
                nc.vector.tensor_tensor(
                    afm8[pp_][ho:ho + 64, 512 * i_:512 * i_ + 512],
                    raw[:], bc[0:64, :], op=MUL)

        # ---- Stage D: proj + residual + LN2 ----
        with ExitStack() as phase3:
            pp_pool = phase3.enter_context(tc.tile_pool(name="pp", bufs=2, space="PSUM"))
            tp2 = phase3.enter_context(tc.tile_pool(name="tp2", bufs=2, space="PSUM"))
            sas = phase3.enter_context(tc.tile_pool(name="sas", bufs=2))
            for j in range(NJ):
                pj = pp_pool.tile([128, C], F32, tag="pp")
                for no, nn in ((0, 512), (512, 256)):
                    for pp in range(NP):
                        lhsT = afm8[pp][:].rearrange(
                            "p (i t) -> p i t", i=2)[:, :, 128 * j:128 * j + 128]
                        rhs = wp8[:, 1536 * pp:1536 * pp + 1536].rearrange(
                            "p (i f) -> p i f", i=2)[:, :, no:no + nn]
                        nc.tensor.matmul(pj[:, no:no + nn], lhsT, rhs,
                                         start=(pp == 0), stop=False,
                                         perf_mode=DR, skip_group_check=True)
                    nc.tensor.matmul(pj[:, no:no + nn], ones[0:1, 0:128],
                                     bp[0:1, no:no + nn],
                                     start=False, stop=True,
                                     skip_group_check=True)
                sa = sas.tile([128, C], F32, tag="sa")
                nc.scalar.activation(sa[:], pj[:], Ident,
                                     scale=1.0 / (S_A * 2 ** 12))
                nc.vector.tensor_tensor(x2[j][:], xo_sb[j][:], sa[:], op=ADD)
                zt = ln_tile(x2[j][:], S_Z2)
                for cp in range(NP):
                    pt = tp2.tile([128, 256], BF16, tag="zt2")
                    for u in range(2):
                        cb = 2 * cp + u
                        nc.tensor.transpose(pt[:, 128 * u:128 * u + 128],
                                            zt[:, 128 * cb:128 * cb + 128],
                                            ident_bf[:])
                    for u in range(2):
                        eng_scalar = (cp + u) % 2 == 1
                        dst = z2f8[cp][:, 512 * u + 128 * j:512 * u + 128 * j + 128]
                        src = pt[:, 128 * u:128 * u + 128]
                        if eng_scalar:
                            nc.scalar.activation(dst, src, Ident)
                        else:
                            nc.vector.tensor_copy(dst, src)
        wpp.release()
        kvat.release()

        # ---- Stage F/G: MLP ----
        outp = tc.alloc_tile_pool(name="outp", bufs=1)
        out_sb = [outp.tile([128, C], F32, tag=f"ou{j}", name=f"ou{j}")
                  for j in range(NJ)]
        with ExitStack() as phase4:
            a1pool = phase4.enter_context(tc.tile_pool(name="a1", bufs=1))
            a18 = [a1pool.tile([128, 1024], FP8, tag=f"a1{pf}", name=f"a1{pf}")
                   for pf in range(NFP)]
            w1st = phase4.enter_context(tc.tile_pool(name="w1st", bufs=8))
            w2st = phase4.enter_context(tc.tile_pool(name="w2st", bufs=3))
            mp_ = phase4.enter_context(tc.tile_pool(name="mp", bufs=3, space="PSUM"))
            fp_ = phase4.enter_context(tc.tile_pool(name="fp", bufs=3, space="PSUM"))
            ftp = phase4.enter_context(tc.tile_pool(name="ftp", bufs=2, space="PSUM"))
            ffs_ = phase4.enter_context(tc.tile_pool(name="ffs", bufs=2))
            for ft in range(NFT):
                w1g = w1st.tile([128, NP * 256], FP8, tag="w1", name=f"w1g{ft}")
                nc.sync.dma_start(w1g[:], w1_d[:, 768 * ft:768 * ft + 768])
                mp = mp_.tile([128, R], F32, tag="mp")
                for pp in range(NP):
                    lhsT = w1g[:, 256 * pp:256 * pp + 256].rearrange(
                        "p (i m) -> p i m", i=2)
                    rhs = z2f8[pp][:].rearrange("p (i t) -> p i t", i=2)
                    nc.tensor.matmul(mp[:], lhsT, rhs,
                                     start=(pp == 0), stop=(pp == NP - 1),
                                     perf_mode=DR, skip_group_check=True)
                nc.scalar.activation(
                    a18[ft // 2][:, 512 * (ft % 2):512 * (ft % 2) + 512],
                    mp[:], Relu, bias=c1[:, ft:ft + 1],
                    scale=S_A1 / (S_Z2 * 2 ** 12))
            for ct in range(NCB):
                w2g = w2st.tile([128, NFP * 256], FP8, tag="w2", name=f"w2g{ct}")
                nc.sync.dma_start(w2g[:], w2_d[:, 3072 * ct:3072 * ct + 3072])
                fp = fp_.tile([128, R], F32, tag="fp")
                for pf in range(NFP):
                    lhsT = w2g[:, 256 * pf:256 * pf + 256].rearrange(
                        "p (i m) -> p i m", i=2)
                    rhs = a18[pf][:].rearrange("p (i t) -> p i t", i=2)
                    nc.tensor.matmul(fp[:], lhsT, rhs,
                                     start=(pf == 0), stop=(pf == NFP - 1),
                                     perf_mode=DR)
                ffs = ffs_.tile([128, R], F32, tag="ffs")
                nc.scalar.activation(ffs[:], fp[:], Relu,
                                     bias=b2c[:, ct:ct + 1],
                                     scale=1.0 / (S_A1 * 2 ** 13))
                for j in range(NJ):
                    pt = ftp.tile([128, 128], F32, tag="ftp")
                    nc.tensor.transpose(pt[:], ffs[:, 128 * j:128 * j + 128],
                                        ident_f[:])
                    nc.vector.tensor_tensor(
                        out_sb[j][:, 128 * ct: 128 * ct + 128],
                        x2[j][:, 128 * ct: 128 * ct + 128],
                        pt[:], op=ADD)

        for j in range(NJ):
            nc.sync.dma_start(out_d[128 * j:128 * j + 128, 0:384],
                              out_sb[j][:, 0:384])
        for j in range(NJ):
            nc.sync.dma_start(out_d[128 * j:128 * j + 128, 384:768],
                              out_sb[j][:, 384:768])
        outp.release()

    nc.finalize()
    return nc


_CACHE = {}


def _get_nc(with_cv=True):
    key = ("nc", with_cv)
    if key not in _CACHE:
        _CACHE[key] = build_program(with_cv=with_cv)
    return _CACHE[key]


def _f8(x, scale):
    import ml_dtypes
    return np.clip(np.asarray(x, np.float32) * scale, -448, 448).astype(
        ml_dtypes.float8_e4m3)


def _pack_lhsT(W, scale):
    """[K, N] -> [128, (N/128)*(K/256)*256] fp8, DoubleRow stationary layout:
    out[p, ct*(K*128/...)...] = W[(2pp+i)*128+p, ct*128+m] ordered (ct, pp, i, m)."""
    K, N = W.shape
    t = W.reshape(K // 256, 2, 128, N // 128, 128).transpose(2, 3, 0, 1, 4)
    return np.ascontiguousarray(t.reshape(128, -1)) * scale


def _pack_moving(W, scale):
    """[K, N] -> [128, (K/256)*2*N] fp8, DoubleRow moving layout:
    out[p, pp*2N + i*N + f] = W[(2pp+i)*128+p, f]."""
    K, N = W.shape
    t = W.reshape(K // 256, 2, 128, N).transpose(2, 0, 1, 3)
    return np.ascontiguousarray(t.reshape(128, -1)) * scale


def _host_prep(inputs):
    import ml_dtypes
    x = np.ascontiguousarray(np.asarray(inputs["x"], np.float32))
    Wq = np.asarray(inputs["Wq"], np.float32).transpose(1, 0, 2).reshape(C, C)
    Wk = np.asarray(inputs["Wk"], np.float32).transpose(1, 0, 2).reshape(C, C)
    Wv = np.asarray(inputs["Wv"], np.float32).transpose(1, 0, 2).reshape(C, C)
    g1 = np.asarray(inputs["ln1_g"], np.float32)
    b1l = np.asarray(inputs["ln1_b"], np.float32)
    g2 = np.asarray(inputs["ln2_g"], np.float32)
    b2l = np.asarray(inputs["ln2_b"], np.float32)
    s = np.float32(C ** -0.5)

    wq = _f8(_pack_lhsT(g1[:, None] * Wq * s, 2 ** 17), 1.0)
    wk = _f8(_pack_lhsT(g1[:, None] * Wk, 2 ** 12), 1.0)
    wv = _f8(_pack_moving(g1[:, None] * Wv, 2 ** 12), 1.0)
    wp = _f8(_pack_moving(np.asarray(inputs["Wp"], np.float32), 2 ** 12), 1.0)
    W1 = np.asarray(inputs["W1"], np.float32)
    w1 = _f8(_pack_lhsT(g2[:, None] * W1, 2 ** 12), 1.0)
    w2 = _f8(_pack_lhsT(np.asarray(inputs["W2"], np.float32), 2 ** 13), 1.0)

    cq = (b1l @ Wq) * s
    ck = b1l @ Wk
    cvt = ((b1l @ Wv) * (S_Z * 2 ** 12)).reshape(2, 384)
    cv = np.zeros((2, 480), np.float32)
    cv[:, 0:384] = cvt
    cv[:, 384::16] = S_Z * 2 ** 12   # -> S_V after the descale copy
    cv = np.ascontiguousarray(cv.reshape(1, 960))
    cqk = np.ascontiguousarray(
        np.concatenate([cq.reshape(NCB, 128).T, ck.reshape(NCB, 128).T], axis=1))
    c1 = np.ascontiguousarray(
        ((b2l @ W1 + np.asarray(inputs["b1"], np.float32)) * S_A1)
        .reshape(NFT, 128).T)
    bp = np.ascontiguousarray(
        (np.asarray(inputs["bp"], np.float32) * (S_A * 2 ** 12)).reshape(1, C))
    b2c = np.ascontiguousarray(
        np.asarray(inputs["b2"], np.float32).reshape(NCB, 128).T)

    in_maps = []
    row_idx = []
    i128 = np.arange(128)
    for core in range(NCORES):
        b, g = core // QUAD, core % QUAD
        # permutation: position 512c + 128u + i -> global row 128((g+u)%4 + 4c) + i
        perm = np.concatenate(
            [128 * (((g + u) % 4) + 4 * c) + i128
             for c in range(4) for u in range(4)])
        own = np.concatenate([np.arange(128 * (g + 4 * j), 128 * (g + 4 * j) + 128)
                              for j in range(NJ)])
        row_idx.append((b, own))
        kl = i128[:, None]
        ql = i128[None, :]
        msk = np.zeros((128, 512), np.float32)
        for u in range(4):
            msk[:, 128 * u:128 * u + 128] = \
                (128 * ((g + u) % 4) + kl <= 128 * g + ql)
        in_maps.append({
            "x_full": np.ascontiguousarray(x[b][perm]),
            "msk": msk.astype(ml_dtypes.float8_e4m3),
            "wq": wq, "wk": wk, "wv": wv, "wp": wp,
            "cqk": cqk, "cv": cv, "bp": bp,
            "w1": w1, "c1": c1, "w2": w2, "b2c": b2c,
            "ones1": np.ones((1, 512), np.float32),
            "c32": np.full((1, 64), S_A, np.float32),
        })
    return in_maps, row_idx


def _run(inputs, trace=False):
    with_cv = bool(np.any(np.asarray(inputs["ln1_b"], np.float32) != 0))
    nc = _get_nc(with_cv=with_cv)
    in_maps, row_idx = _host_prep(inputs)
    res = run_bass_kernel_spmd(nc, in_maps, core_ids=list(range(NCORES)),
                               trace=trace)
    out = np.zeros((B, T, C), np.float32)
    for core in range(NCORES):
        b, rows = row_idx[core]
        out[b][rows] = res.results[core]["out"]
    return out, res


def kernel(**inputs):
    out, _ = _run(inputs, trace=False)
    return out
